# revision 12
# baseline (speedup 1.0000x reference)
"""LIF (leaky integrate-and-fire) spiking-activation kernel for Trainium2.

Full input x: [4, 64, 128, 32, 32] fp32.  Recurrence over t (STEP=4):
    mem   = mem * 0.25 + x[t]
    spike = (mem > 0.5)            # forward value of the surrogate-grad trick
    out[t] = spike (as 1.0 / 0.0)
    mem   = mem * (1 - spike)      # reset where spiked

The reference's straight-through expression stop_grad(out_s - out_bp) + out_bp
is numerically exactly out_s in fp32 (for b in [0,1], (1-b)+b rounds to 1.0 and
(0-b)+b to 0.0), so the forward pass is just exact thresholding + reset.

Sharding: batch dim (64) split across 8 NeuronCores, 8 batches each.
Per core the per-timestep slab is 8*128*32*32 = 1M elements, viewed as
4 chunks of [128 partitions, 2048 free] (each chunk 1 MiB contiguous in DRAM).

Engine split per (t, chunk):
  ACT   : spike = Sign(mem - 0.5)            -> int8 {-1,0,1}   (exact)
  DVE   : masked = (mem is_le 0.5) mult mem  (reset, 1 op)
          mem'   = (masked mult 0.25) add x  (decay+integrate, 1 op)
  HWDGE : x loads (prefetched one step ahead, 6-deep pool), spike stores
          (int8, 4x fewer output bytes than fp32)
Host converts int8 sign output to {0.0, 1.0} fp32 (s > 0), which also handles
the measure-zero mem == 0.5 case exactly like the reference (sign(0)=0 -> 0.0).
"""

import numpy as np

STEP = 4
NCORES = 8
B_PER_CORE = 8
NCHUNK = 4
P = 128
CF = 2048  # free-dim per chunk; NCHUNK*P*CF == 8*128*32*32

_CACHE = {}


def _build():
    if "nc" in _CACHE:
        return _CACHE["nc"]

    from contextlib import ExitStack

    import concourse.bass as bass
    import concourse.tile as tile
    from concourse import bacc, mybir

    f32 = mybir.dt.float32
    i8 = mybir.dt.int8
    Alu = mybir.AluOpType

    nc = bacc.Bacc(
        "TRN2",
        target_bir_lowering=False,
        debug=False,
        enable_asserts=False,
        num_devices=NCORES,
    )
    x_h = nc.dram_tensor("x", [STEP, NCHUNK, P, CF], f32, kind="ExternalInput")
    s_h = nc.dram_tensor("spikes", [STEP, NCHUNK, P, CF], i8, kind="ExternalOutput")
    x_ap = x_h.ap()
    s_ap = s_h.ap()

    with ExitStack() as ctx:
        tc = ctx.enter_context(tile.TileContext(nc))
        mem_pool = ctx.enter_context(tc.tile_pool(name="mem", bufs=1))
        x_pool = ctx.enter_context(tc.tile_pool(name="xin", bufs=8))
        spk_pool = ctx.enter_context(tc.tile_pool(name="spk", bufs=8))
        const_pool = ctx.enter_context(tc.tile_pool(name="const", bufs=1))

        # per-partition bias vector holding -0.5 for the Sign activation
        neg_half = const_pool.tile([P, 1], f32, name="neg_half")
        nc.gpsimd.memset(neg_half[:], -0.5)

        # Persistent ping-pong state tiles, one pair per chunk.
        memA = [mem_pool.tile([P, CF], f32, tag=f"a{j}", name=f"memA{j}") for j in range(NCHUNK)]
        memB = [mem_pool.tile([P, CF], f32, tag=f"b{j}", name=f"memB{j}") for j in range(NCHUNK)]

        # t = 0: mem is just x[0]
        for j in range(NCHUNK):
            nc.sync.dma_start(out=memA[j][:], in_=x_ap[0, j])

        for t in range(STEP):
            cur = memA if t % 2 == 0 else memB
            nxt = memB if t % 2 == 0 else memA
            xts = []
            if t < STEP - 1:
                for j in range(NCHUNK):
                    xt = x_pool.tile([P, CF], f32, tag="x", name=f"x{t}_{j}")
                    nc.sync.dma_start(out=xt[:], in_=x_ap[t + 1, j])
                    xts.append(xt)
            for j in range(NCHUNK):
                # serial backbone first: the t-chain state update on DVE
                if t < STEP - 1:
                    xt = xts[j]
                    # reset: nxt = (cur <= 0.5) * cur
                    nc.vector.scalar_tensor_tensor(
                        out=nxt[j][:], in0=cur[j][:], scalar=0.5, in1=cur[j][:],
                        op0=Alu.is_le, op1=Alu.mult,
                    )
                    # integrate: nxt = nxt * 0.25 + x[t+1]
                    nc.vector.scalar_tensor_tensor(
                        out=nxt[j][:], in0=nxt[j][:], scalar=0.25, in1=xt[:],
                        op0=Alu.mult, op1=Alu.add,
                    )
                spk = spk_pool.tile([P, CF], i8, tag="spk", name=f"spk{t}_{j}")
                # spike = sign(mem - 0.5): exact threshold, int8 out
                nc.scalar.sign(spk[:], cur[j][:], bias=neg_half[:])
                nc.sync.dma_start(out=s_ap[t, j], in_=spk[:])

    nc.compile()
    _CACHE["nc"] = nc
    return nc


def kernel(x):
    nc = _build()
    from concourse.bass_utils import run_bass_kernel_spmd

    x = np.asarray(x)
    assert x.shape == (STEP, 64, 128, 32, 32) and x.dtype == np.float32

    in_maps = [
        {
            "x": np.ascontiguousarray(x[:, i * B_PER_CORE:(i + 1) * B_PER_CORE])
            .reshape(STEP, NCHUNK, P, CF)
        }
        for i in range(NCORES)
    ]
    trace = bool(_CACHE.get("trace"))
    res = run_bass_kernel_spmd(
        nc, in_maps, core_ids=list(range(NCORES)), trace=trace
    )
    _CACHE["last_result"] = res

    out = np.empty((STEP, 64, 128, 32, 32), np.float32)
    for i, r in enumerate(res.results):
        s = r["spikes"]  # int8 [STEP, NCHUNK, P, CF]
        out[:, i * B_PER_CORE:(i + 1) * B_PER_CORE] = (
            (s > 0).astype(np.float32).reshape(STEP, B_PER_CORE, 128, 32, 32)
        )
    return out


# revision 13
# speedup vs baseline: 1.0523x; 1.0523x over previous
"""LIF (leaky integrate-and-fire) spiking-activation kernel for Trainium2.

Full input x: [4, 64, 128, 32, 32] fp32.  Recurrence over t (STEP=4):
    mem   = mem * 0.25 + x[t]
    spike = (mem > 0.5)            # forward value of the surrogate-grad trick
    out[t] = spike (as 1.0 / 0.0)
    mem   = mem * (1 - spike)      # reset where spiked

The reference's straight-through expression stop_grad(out_s - out_bp) + out_bp
is numerically exactly out_s in fp32 (for b in [0,1], (1-b)+b rounds to 1.0 and
(0-b)+b to 0.0), so the forward pass is just exact thresholding + reset.

Sharding: batch dim (64) split across 8 NeuronCores, 8 batches each.
Per core the per-timestep slab is 8*128*32*32 = 1M elements, viewed as
4 chunks of [128 partitions, 2048 free] (each chunk 1 MiB contiguous in DRAM).

Engine split per (t, chunk):
  ACT   : spike = Sign(mem - 0.5)            -> int8 {-1,0,1}   (exact)
  DVE   : masked = (mem is_le 0.5) mult mem  (reset, 1 op)
          mem'   = (masked mult 0.25) add x  (decay+integrate, 1 op)
  HWDGE : x loads (prefetched one step ahead, 6-deep pool), spike stores
          (int8, 4x fewer output bytes than fp32)
Host converts int8 sign output to {0.0, 1.0} fp32 (s > 0), which also handles
the measure-zero mem == 0.5 case exactly like the reference (sign(0)=0 -> 0.0).
"""

import numpy as np

STEP = 4
NCORES = 8
B_PER_CORE = 8
NCHUNK = 4
P = 128
CF = 2048  # free-dim per chunk; NCHUNK*P*CF == 8*128*32*32

_CACHE = {}


def _build():
    if "nc" in _CACHE:
        return _CACHE["nc"]

    from contextlib import ExitStack

    import concourse.bass as bass
    import concourse.tile as tile
    from concourse import bacc, mybir

    f32 = mybir.dt.float32
    i8 = mybir.dt.int8
    Alu = mybir.AluOpType

    nc = bacc.Bacc(
        "TRN2",
        target_bir_lowering=False,
        debug=False,
        enable_asserts=False,
        num_devices=NCORES,
    )
    x_h = nc.dram_tensor("x", [STEP, NCHUNK, P, CF], f32, kind="ExternalInput")
    s_h = nc.dram_tensor("spikes", [STEP, NCHUNK, P, CF], i8, kind="ExternalOutput")
    x_ap = x_h.ap()
    s_ap = s_h.ap()

    with ExitStack() as ctx:
        tc = ctx.enter_context(tile.TileContext(nc))
        mem_pool = ctx.enter_context(tc.tile_pool(name="mem", bufs=1))
        x_pool = ctx.enter_context(tc.tile_pool(name="xin", bufs=6))
        spk_pool = ctx.enter_context(tc.tile_pool(name="spk", bufs=8))
        const_pool = ctx.enter_context(tc.tile_pool(name="const", bufs=1))

        # per-partition bias vector holding -0.5 for the Sign activation
        neg_half = const_pool.tile([P, 1], f32, name="neg_half")
        nc.gpsimd.memset(neg_half[:], -0.5)

        # Persistent ping-pong state tiles, one pair per chunk.
        memA = [mem_pool.tile([P, CF], f32, tag=f"a{j}", name=f"memA{j}") for j in range(NCHUNK)]
        memB = [mem_pool.tile([P, CF], f32, tag=f"b{j}", name=f"memB{j}") for j in range(NCHUNK)]

        # t = 0: mem is just x[0]
        for j in range(NCHUNK):
            nc.sync.dma_start(out=memA[j][:], in_=x_ap[0, j])

        for t in range(STEP):
            cur = memA if t % 2 == 0 else memB
            nxt = memB if t % 2 == 0 else memA
            xts = []
            if t < STEP - 1:
                for j in range(NCHUNK):
                    xt = x_pool.tile([P, CF], f32, tag="x", name=f"x{t}_{j}")
                    nc.sync.dma_start(out=xt[:], in_=x_ap[t + 1, j])
                    xts.append(xt)
            for j in range(NCHUNK):
                spk = spk_pool.tile([P, CF], i8, tag="spk", name=f"spk{t}_{j}")
                # spike = sign(mem - 0.5): exact threshold, int8 out
                nc.scalar.sign(spk[:], cur[j][:], bias=neg_half[:])
                nc.sync.dma_start(out=s_ap[t, j], in_=spk[:])
                if t < STEP - 1:
                    xt = xts[j]
                    # reset: nxt = (cur <= 0.5) * cur
                    nc.vector.scalar_tensor_tensor(
                        out=nxt[j][:], in0=cur[j][:], scalar=0.5, in1=cur[j][:],
                        op0=Alu.is_le, op1=Alu.mult,
                    )
                    # integrate: nxt = nxt * 0.25 + x[t+1]
                    nc.vector.scalar_tensor_tensor(
                        out=nxt[j][:], in0=nxt[j][:], scalar=0.25, in1=xt[:],
                        op0=Alu.mult, op1=Alu.add,
                    )

    nc.compile()
    _CACHE["nc"] = nc
    return nc


def kernel(x):
    nc = _build()
    from concourse.bass_utils import run_bass_kernel_spmd

    x = np.asarray(x)
    assert x.shape == (STEP, 64, 128, 32, 32) and x.dtype == np.float32

    in_maps = [
        {
            "x": np.ascontiguousarray(x[:, i * B_PER_CORE:(i + 1) * B_PER_CORE])
            .reshape(STEP, NCHUNK, P, CF)
        }
        for i in range(NCORES)
    ]
    trace = bool(_CACHE.get("trace"))
    res = run_bass_kernel_spmd(
        nc, in_maps, core_ids=list(range(NCORES)), trace=trace
    )
    _CACHE["last_result"] = res

    out = np.empty((STEP, 64, 128, 32, 32), np.float32)
    for i, r in enumerate(res.results):
        s = r["spikes"]  # int8 [STEP, NCHUNK, P, CF]
        out[:, i * B_PER_CORE:(i + 1) * B_PER_CORE] = (
            (s > 0).astype(np.float32).reshape(STEP, B_PER_CORE, 128, 32, 32)
        )
    return out
